# revision 15
# baseline (speedup 1.0000x reference)
"""FAVOR+ (Performer) attention kernel for Trainium2, 8 NeuronCores.

Problem: nn_Attention_4810363372688
  B=4, L=4096, HID=1024, H=16, DH=64, M=128, noncausal relu-kernel FAVOR+.

Sharding: core c handles batch b=c//2 and head-group hg=c%2 (8 heads).
Each core computes a partial output (its 8 heads' contribution to the
output projection) in transposed layout [HID, L] (bf16); the host sums the
two head-group partials per batch and transposes back. No collectives.

Host prep folds proj and the 1/sqrt(M) ratio into the K-side weight
(Wkp = ratio * Wk @ proj.T) so kp is a single fused matmul, and
pre-transposes/casts x to bf16 [HID, L].

Pass-2 normalization path: qp carries +eps (DVE max/add evac, exact
num/den), den for 4 heads is computed by K=128 matmuls into quadrant rows
{0,32,64,96} of one PSUM bank, reciprocal via the fast custom-DVE approx,
and the per-head recip row is broadcast across 64 partitions with rank-1
PE matmuls so att = num * bc is a single tensor_mul per head pair.
"""

import os
import numpy as np
import ml_dtypes

import concourse.bass as bass
import concourse.mybir as mybir
import concourse.tile as tile
from concourse import bacc
from concourse import bass_utils

B, L, HID, H, DH, M = 4, 4096, 1024, 16, 64, 128
EPS = 1e-3
HC = H // 2          # heads per core = 8
KT = HID // 128      # 8 contraction k-tiles
LT = L // 128        # 32 token tiles (pass 1)
NCH = L // 512       # 8 L-chunks (pass 2)

BF16 = mybir.dt.bfloat16
F32 = mybir.dt.float32
FP8 = mybir.dt.float8e4
DR = mybir.MatmulPerfMode.DoubleRow
WS = 16.0  # fp8 weight scale (folded into wkp and the kp eps; cancels in num/den)

_cache = {}

# exec time of the most recent run (ns), when KERNEL_TRACE=1
last_exec_time_ns = None
last_trace_path = None


def build_kernel():
    nc = bacc.Bacc("TRN2", target_bir_lowering=False, debug=False, num_devices=1)

    xsT = nc.dram_tensor("xsT", [HID, L], FP8, kind="ExternalInput").ap()
    xsb = nc.dram_tensor("xsb", [HID, L], BF16, kind="ExternalInput").ap()
    xqT = nc.dram_tensor("xqT", [HID, L], BF16, kind="ExternalInput").ap()
    wkp8 = nc.dram_tensor("wkp8", [HID, HC * M], FP8, kind="ExternalInput").ap()
    wvb = nc.dram_tensor("wvb", [HID, HC * DH], BF16, kind="ExternalInput").ap()
    wq = nc.dram_tensor("wq", [HID, HC * DH], BF16, kind="ExternalInput").ap()
    wo = nc.dram_tensor("wo", [HC * DH, HID], BF16, kind="ExternalInput").ap()
    projt = nc.dram_tensor("projt", [DH, M], BF16, kind="ExternalInput").ap()
    outT = nc.dram_tensor("outT", [HID, L], F32, kind="ExternalOutput").ap()

    # DRAM views with the 128-partition k-tile split explicit
    xsT_r = xsT.rearrange("(k p) l -> p k l", p=128)     # [128, 8, L]
    xsb_r = xsb.rearrange("(k p) l -> p k l", p=128)     # [128, 8, L]
    xqT_r = xqT.rearrange("(k p) l -> p k l", p=128)     # [128, 8, L]
    wkp_r = wkp8.rearrange("(k p) f -> p k f", p=128)    # [128, 8, 1024]
    wvb_r = wvb.rearrange("(k p) f -> p k f", p=128)     # [128, 8, 512]
    wq_r = wq.rearrange("(k p) f -> p k f", p=128)       # [128, 8, 512]
    wo_r = wo.rearrange("(t p) o -> p t o", p=128)       # [128, 4, 1024]

    with tile.TileContext(nc) as tc:
        with tc.tile_pool(name="singles", bufs=1) as singles:
            # split per k-tile across both DGE rings so the first matmuls
            # only wait on slice 0 and the rings fill in parallel
            wkp_sb = singles.tile([128, KT, HC * M], FP8)
            wv_sb = singles.tile([128, KT, HC * DH], BF16)
            for k in range(KT):
                nc.scalar.dma_start(out=wkp_sb[:, k, :], in_=wkp_r[:, k, :])
                nc.scalar.dma_start(out=wv_sb[:, k, :], in_=wvb_r[:, k, :])
            # pass-2 weights queue behind wkv on the scalar ring; they are
            # not needed until pass 2
            wq_sb = singles.tile([128, KT, HC * DH], BF16)
            wo_sb = singles.tile([128, 4, HID], BF16)
            # proj.T duplicated into both partition halves: the qp matmul for
            # odd heads then runs entirely at base-64 (row-tiling), letting
            # the qT evacuation be a single shift-free ACT copy per m-tile
            projt_sb = singles.tile([128, M], BF16)
            kvsks_sb = singles.tile([128, HC, DH + 1], BF16)
            # ks columns replicated 64-wide: den matmuls then produce the
            # denominator already broadcast across each head's 64 partitions
            ks64_sb = singles.tile([128, HC, DH], BF16)

            # xq tiles live across both passes so the first chunks can be
            # prefetched on the scalar ring while pass 1 computes
            xq_pool = tc.alloc_tile_pool(name="xqp", bufs=3)
            xq_tiles = {}
            for ch in range(2):
                xq_tiles[ch] = xq_pool.tile([128, KT, 512], BF16, tag="xq", name=f"xq{ch}")

            # ---------------- pass 1: kp, v -> kvs/ks accumulation ----------
            with (
                tc.tile_pool(name="p1s", bufs=2) as p1s,
                tc.tile_pool(name="p1ps", bufs=2, space="PSUM") as p1ps,
                tc.tile_pool(name="kvps", bufs=1, space="PSUM") as kvps,
            ):
                kv_ps_lo = kvps.tile([128, 4 * (DH + 1)], F32, tag="kvlo")
                kv_ps_hi = kvps.tile([128, 4 * (DH + 1)], F32, tag="kvhi")

                for li in range(LT):
                    xs = p1s.tile([128, KT, 128], FP8, tag="xs", bufs=8)
                    nc.sync.dma_start(
                        out=xs, in_=xsT_r[:, :, li * 128 : (li + 1) * 128]
                    )
                    xsv = p1s.tile([128, KT, 128], BF16, tag="xsv", bufs=8)
                    nc.sync.dma_start(
                        out=xsv, in_=xsb_r[:, :, li * 128 : (li + 1) * 128]
                    )
                    kp_ps_a = p1ps.tile([128, 512], F32, tag="kpa")
                    kp_ps_b = p1ps.tile([128, 512], F32, tag="kpb")
                    v_ps = p1ps.tile([128, 512], F32, tag="vps")
                    for k in range(0, KT, 2):
                        st, sp = k == 0, k == KT - 2
                        lhs = xs[:, k : k + 2, :]
                        nc.tensor.matmul(
                            kp_ps_a, lhsT=lhs, rhs=wkp_sb[:, k : k + 2, 0:512],
                            start=st, stop=sp, perf_mode=DR,
                        )
                        nc.tensor.matmul(
                            kp_ps_b, lhsT=lhs, rhs=wkp_sb[:, k : k + 2, 512:1024],
                            start=st, stop=sp, perf_mode=DR,
                        )
                        nc.tensor.matmul(
                            v_ps, lhsT=xsv[:, k, :], rhs=wv_sb[:, k, :],
                            start=st, stop=False,
                        )
                        nc.tensor.matmul(
                            v_ps, lhsT=xsv[:, k + 1, :], rhs=wv_sb[:, k + 1, :],
                            start=False, stop=sp,
                        )
                    # kp = max(x@Wkp, 0) + eps, cast bf16
                    kp_sb = p1s.tile([128, HC, 128], BF16, tag="kp")
                    nc.vector.tensor_scalar(
                        kp_sb[:, 0:4, :],
                        kp_ps_a.rearrange("p (a b) -> p a b", a=4), 0.0, WS * EPS,
                        op0=mybir.AluOpType.max, op1=mybir.AluOpType.add,
                    )
                    nc.vector.tensor_scalar(
                        kp_sb[:, 4:8, :],
                        kp_ps_b.rearrange("p (a b) -> p a b", a=4), 0.0, WS * EPS,
                        op0=mybir.AluOpType.max, op1=mybir.AluOpType.add,
                    )
                    # v with a ones column appended per head: [128, 8, 65]
                    v_sb = p1s.tile([128, HC, DH + 1], BF16, tag="v")
                    nc.scalar.copy(
                        v_sb[:, :, 0:DH],
                        v_ps.rearrange("p (h d) -> p h d", h=HC),
                    )
                    nc.gpsimd.memset(v_sb[:, :, DH : DH + 1], 1.0)
                    # pass-2 weights/inputs paced onto the gpsimd ring once
                    # the pass-1 input stream is in steady state
                    if li == 6:
                        nc.gpsimd.dma_start(out=wq_sb, in_=wq_r)
                    elif li == 8:
                        nc.gpsimd.dma_start(
                            out=wo_sb[:, 0:2, :], in_=wo_r[:, 0:2, :]
                        )
                    elif li == 10:
                        nc.gpsimd.dma_start(
                            out=wo_sb[:, 2:4, :], in_=wo_r[:, 2:4, :]
                        )
                    elif li == 12:
                        nc.gpsimd.dma_start(out=projt_sb[0:DH, :], in_=projt)
                        nc.gpsimd.dma_start(out=projt_sb[DH:128, :], in_=projt)
                        nc.gpsimd.dma_start(
                            out=xq_tiles[0], in_=xqT_r[:, :, 0:512]
                        )
                    elif li == 14:
                        nc.gpsimd.dma_start(
                            out=xq_tiles[1], in_=xqT_r[:, :, 512:1024]
                        )
                    # One accumulation group per PSUM bank: start only on the
                    # very first MM touching the bank (start marks the whole
                    # 2KB zero-region pending-zero; later heads' first writes
                    # land on pending bytes and overwrite, then accumulate).
                    for h in range(HC):
                        ps = kv_ps_lo if h < 4 else kv_ps_hi
                        j = h % 4
                        nc.tensor.matmul(
                            ps[:, j * (DH + 1) : (j + 1) * (DH + 1)],
                            lhsT=kp_sb[:, h, :],
                            rhs=v_sb[:, h, :],
                            start=(li == 0 and j == 0),
                            stop=(li == LT - 1 and j == 3),
                        )

                nc.scalar.copy(
                    kvsks_sb[:, 0:4, :],
                    kv_ps_lo.rearrange("p (h d) -> p h d", h=4),
                )
                nc.vector.tensor_copy(
                    out=kvsks_sb[:, 4:8, :],
                    in_=kv_ps_hi.rearrange("p (h d) -> p h d", h=4),
                )
                for h in range(HC):
                    nc.vector.tensor_copy(
                        out=ks64_sb[:, h, :],
                        in_=kvsks_sb[:, h, DH : DH + 1].broadcast_to([128, DH]),
                    )

            # ---------------- pass 2: q -> qp -> num/den -> att -> out ------
            with (
                tc.tile_pool(name="p2s", bufs=2) as p2s,
                tc.tile_pool(name="p2ps", bufs=2, space="PSUM") as p2ps,
            ):
                def emit_out(oatt):
                    for j in range(8):
                        out_ps = p2ps.tile([128, 512], F32, tag="out", name="out_ps")
                        for t in range(4):
                            nc.tensor.matmul(
                                out_ps,
                                lhsT=wo_sb[:, t, j * 128 : (j + 1) * 128],
                                rhs=oatt[:, t, :],
                                start=(t == 0), stop=(t == 3),
                            )
                        out_sb = p2s.tile([128, 512], F32, tag="outsb", name="out_sb")
                        if j % 4 == 3:
                            nc.vector.tensor_copy(out=out_sb, in_=out_ps)
                        else:
                            nc.scalar.copy(out_sb, out_ps)
                        nc.scalar.dma_start(
                            out=outT[j * 128 : (j + 1) * 128, osl], in_=out_sb
                        )

                for ch in range(NCH):
                    lsl = slice(ch * 512, (ch + 1) * 512)
                    osl = lsl
                    if ch + 2 < NCH:
                        nxt = ch + 2
                        xq_tiles[nxt] = xq_pool.tile([128, KT, 512], BF16, tag="xq", name=f"xq{nxt}")
                        nc.sync.dma_start(
                            out=xq_tiles[nxt],
                            in_=xqT_r[:, :, nxt * 512 : (nxt + 1) * 512],
                        )
                    xq = xq_tiles[ch]
                    # qT [hd, l] per m-tile; heads (2m, 2m+1) stay stacked in
                    # the two partition halves (single shift-free ACT copy)
                    qt_sb = p2s.tile([128, 4, 512], BF16, tag="qt")
                    for m in range(4):
                        qt_ps = p2ps.tile([128, 512], F32, tag="proj", name="qt_ps")
                        for k in range(KT):
                            nc.tensor.matmul(
                                qt_ps,
                                lhsT=wq_sb[:, k, m * 128 : (m + 1) * 128],
                                rhs=xq[:, k, :],
                                start=(k == 0), stop=(k == KT - 1),
                            )
                        nc.vector.tensor_copy(out=qt_sb[:, m, :], in_=qt_ps)
                    # qp = max(ratio*(q@proj.T), 0) + eps  (ratio folded in
                    # projt); odd heads run at base-64 via row-tiling.
                    # qp carries the +eps so num and den are exact.
                    qp_sb = p2s.tile([128, HC, 512], BF16, tag="qp")
                    for h in range(HC):
                        qp_ps = p2ps.tile([128, 512], F32, tag="proj", name="qp_ps")
                        hp = (h % 2) * 64
                        nc.tensor.matmul(
                            qp_ps,
                            lhsT=projt_sb[hp : hp + DH, :],
                            rhs=qt_sb[hp : hp + DH, h // 2, :],
                            start=True, stop=True,
                        )
                        if h % 2 == 0:
                            nc.scalar.activation(
                                qp_sb[:, h, :], qp_ps,
                                mybir.ActivationFunctionType.Relu,
                            )
                        else:
                            nc.vector.tensor_scalar_max(qp_sb[:, h, :], qp_ps, 0.0)
                    # per head pair: den replicated across the pair's 64+64
                    # partitions by the ks64 matmuls, fast-recip to SBUF, then
                    # att = num * recip in a single tensor_tensor per pair
                    att_sb = p2s.tile([128, 4, 512], BF16, tag="att")
                    for p in range(4):
                        den_ps = p2ps.tile([128, 512], F32, tag="denps", name=f"den{p}")
                        num_ps = p2ps.tile([128, 512], F32, tag="num", name=f"num{p}")
                        for i in range(2):
                            h = 2 * p + i
                            nc.tensor.matmul(
                                den_ps[64 * i : 64 * i + 64, :],
                                lhsT=ks64_sb[:, h, :],
                                rhs=qp_sb[:, h, :],
                                start=True, stop=True,
                                tile_position=(0, 64 * i),
                            )
                        denr_sb = p2s.tile(
                            [128, 512], F32, tag="denr", name=f"denr{p}"
                        )
                        nc.vector.reciprocal_approx_fast(out=denr_sb, in_=den_ps)
                        for i in range(2):
                            h = 2 * p + i
                            nc.tensor.matmul(
                                num_ps[64 * i : 64 * i + 64, :],
                                lhsT=kvsks_sb[:, h, 0:DH],
                                rhs=qp_sb[:, h, :],
                                start=True, stop=True,
                                tile_position=(0, 64 * i),
                            )
                        nc.vector.tensor_tensor(
                            out=att_sb[:, p, :], in0=num_ps, in1=denr_sb,
                            op=mybir.AluOpType.mult,
                        )
                    emit_out(att_sb)
            xq_pool.release()

    nc.compile()
    return nc


def _prep_inputs(query_input, source_input, Wq, Wk, Wv, Wo, proj):
    """Host-side shard + layout prep. Returns in_maps for 8 cores."""
    bf = ml_dtypes.bfloat16
    f8 = ml_dtypes.float8_e4m3
    WS = 16.0
    ratio = 1.0 / float(np.sqrt(M))
    # fused K-side weight: ratio * Wk @ proj.T -> [HID, H, M], x16 for fp8
    wkp = WS * ratio * np.einsum("dhk,mk->dhm", Wk, proj)
    projt_all = (ratio * proj.T).astype(bf)  # [DH, M]
    in_maps = []
    for c in range(8):
        b, hg = c // 2, c % 2
        hs = slice(hg * HC, (hg + 1) * HC)
        xs_t = np.ascontiguousarray(source_input[b].T)
        wkp_c = wkp[:, hs, :].reshape(HID, HC * M)
        in_maps.append(
            {
                "xsT": xs_t.astype(f8),
                "xsb": xs_t.astype(bf),
                "xqT": np.ascontiguousarray(query_input[b].T).astype(bf),
                "wkp8": wkp_c.astype(f8),
                "wvb": Wv[:, hs, :].reshape(HID, HC * DH).astype(bf),
                "wq": Wq[:, hs, :].reshape(HID, HC * DH).astype(bf),
                "wo": Wo[hs].reshape(HC * DH, HID).astype(bf),
                "projt": projt_all,
            }
        )
    return in_maps


def kernel(query_input, source_input, Wq, Wk, Wv, Wo, proj, training=0):
    global last_exec_time_ns, last_trace_path
    query_input = np.asarray(query_input, dtype=np.float32)
    source_input = np.asarray(source_input, dtype=np.float32)
    Wq = np.asarray(Wq, dtype=np.float32)
    Wk = np.asarray(Wk, dtype=np.float32)
    Wv = np.asarray(Wv, dtype=np.float32)
    Wo = np.asarray(Wo, dtype=np.float32)
    proj = np.asarray(proj, dtype=np.float32)

    if "nc" not in _cache:
        _cache["nc"] = build_kernel()
    nc = _cache["nc"]

    in_maps = _prep_inputs(query_input, source_input, Wq, Wk, Wv, Wo, proj)

    trace = os.environ.get("KERNEL_TRACE", "0") == "1"
    kwargs = {}
    if trace:
        try:
            import profhook

            profhook.install()
            kwargs["trace"] = True
            kwargs["trace_cores"] = [0]
        except Exception:
            pass
    res = bass_utils.run_bass_kernel_spmd(
        nc, in_maps, core_ids=list(range(8)), **kwargs
    )
    if trace:
        last_exec_time_ns = res.exec_time_ns
        if res.instructions_and_trace is not None:
            last_trace_path = res.instructions_and_trace[1]

    out = np.empty((B, L, HID), dtype=np.float32)
    for b in range(B):
        acc = res.results[2 * b]["outT"].astype(np.float32) + res.results[
            2 * b + 1
        ]["outT"].astype(np.float32)
        out[b] = acc.T
    return out
